# revision 6
# baseline (speedup 1.0000x reference)
"""NTM-style memory module (scatter_memory) on 8 TRN2 NeuronCores.

Sharding: pure data-parallel over batch. B=1024 rows -> 128 rows/core,
batch rows live on SBUF partitions (128 partitions = 128 rows).

Per core (b on partitions everywhere, free axis = n or m):
  phase 1: stream memory slabs, compute
           num[b,n]   = sum_m mem[b,n,m]*key[b,m]   (DVE tensor_tensor_reduce)
           norms2[b,n]= sum_m mem[b,n,m]^2          (ACT Square + accum_out)
  chain:   cosine -> softmax(beta*cos) -> gate -> circular shift -> sharpen
           (all [128,512] free-axis ops, DVE+ACT)
  phase 2: stream memory slabs again,
           r[b,m]  += w[b,n]*mem[b,n,m]             (DVE scalar_tensor_tensor)
           F = 1 - w_n*e                            (DVE tensor_scalar, 2x mode)
           v = mem * F                              (DVE tensor_tensor slab)
           out_n = v + w_n*a                        (DVE scalar_tensor_tensor)
  out = concat[w (512), r (256), new_mem (131072)] per row.
"""

import os
import numpy as np
from contextlib import ExitStack

B, N, M = 128, 512, 256          # per-core shard: batch rows, locations, vec
NCORES = 8
SLAB = 16                        # n's per streamed slab
NSLABS = N // SLAB
OUT_COLS = N + M + N * M         # 131840
EPS_COS = 1e-8
EPS_ADD = 1e-16

LAST_RESULTS = None              # BassKernelResults of the most recent run


def _build():
    import concourse.bass as bass  # noqa: F401
    import concourse.tile as tile
    from concourse import bacc, mybir

    f32 = mybir.dt.float32
    AL = mybir.AluOpType
    AF = mybir.ActivationFunctionType

    nc = bacc.Bacc("TRN2", target_bir_lowering=False, debug=False,
                   num_devices=NCORES)

    mem_d = nc.dram_tensor("memory", [B, N, M], f32, kind="ExternalInput")
    key_d = nc.dram_tensor("key", [B, M], f32, kind="ExternalInput")
    beta_d = nc.dram_tensor("beta", [B, 1], f32, kind="ExternalInput")
    g_d = nc.dram_tensor("g", [B, 1], f32, kind="ExternalInput")
    s_d = nc.dram_tensor("s", [B, 3], f32, kind="ExternalInput")
    gamma_d = nc.dram_tensor("gamma", [B, 1], f32, kind="ExternalInput")
    wprev_d = nc.dram_tensor("w_prev", [B, N], f32, kind="ExternalInput")
    e_d = nc.dram_tensor("e", [B, M], f32, kind="ExternalInput")
    a_d = nc.dram_tensor("a", [B, M], f32, kind="ExternalInput")
    out_d = nc.dram_tensor("out", [B, OUT_COLS], f32, kind="ExternalOutput")

    with tile.TileContext(nc) as tc, ExitStack() as ctx:
        singles = ctx.enter_context(tc.tile_pool(name="singles", bufs=1))
        mems = ctx.enter_context(tc.tile_pool(name="mems", bufs=3))
        fpool = ctx.enter_context(tc.tile_pool(name="fpool", bufs=2))
        scr = ctx.enter_context(tc.tile_pool(name="scr", bufs=2))

        # --- small resident tiles ------------------------------------------
        k_sb = singles.tile([B, M], f32)
        nc.sync.dma_start(k_sb[:], key_d[:, :])
        e_sb = singles.tile([B, M], f32)
        nc.sync.dma_start(e_sb[:], e_d[:, :])
        a_sb = singles.tile([B, M], f32)
        nc.sync.dma_start(a_sb[:], a_d[:, :])
        wprev_sb = singles.tile([B, N], f32)
        nc.sync.dma_start(wprev_sb[:], wprev_d[:, :])
        beta_sb = singles.tile([B, 1], f32)
        nc.sync.dma_start(beta_sb[:], beta_d[:, :])
        g_sb = singles.tile([B, 1], f32)
        nc.sync.dma_start(g_sb[:], g_d[:, :])
        s_sb = singles.tile([B, 3], f32)
        nc.sync.dma_start(s_sb[:], s_d[:, :])
        gamma_sb = singles.tile([B, 1], f32)
        nc.sync.dma_start(gamma_sb[:], gamma_d[:, :])

        num_sb = singles.tile([B, N], f32)
        norms2_sb = singles.tile([B, N], f32)

        # k replicated SLAB times along free dim, for slab-wide products
        k_rep = singles.tile([B, SLAB, M], f32)
        for t in range(SLAB):
            nc.vector.tensor_copy(k_rep[:, t, :], k_sb[:])

        # --- phase 1: num + norms ------------------------------------------
        # (tensor_tensor_reduce / activation accum_out fault this runtime --
        #  use slab product + segmented tensor_reduce instead)
        for j in range(NSLABS):
            ms = mems.tile([B, SLAB, M], f32, tag="mem")
            nc.sync.dma_start(ms[:], mem_d[:, j * SLAB:(j + 1) * SLAB, :])
            us = scr.tile([B, SLAB, M], f32, tag="us")
            nc.vector.tensor_tensor(us[:], ms[:], k_rep[:], AL.mult)
            nc.vector.reduce_sum(num_sb[:, j * SLAB:(j + 1) * SLAB], us[:],
                                 axis=mybir.AxisListType.X)
            sq = scr.tile([B, SLAB, M], f32, tag="us")
            nc.scalar.activation(sq[:], ms[:], AF.Square)
            nc.vector.reduce_sum(norms2_sb[:, j * SLAB:(j + 1) * SLAB], sq[:],
                                 axis=mybir.AxisListType.X)

        # --- chain: cosine -> softmax -> gate -> shift -> sharpen ----------
        # key norm
        ksq = scr.tile([B, M], f32, tag="tts")
        k2 = singles.tile([B, 1], f32)
        nc.scalar.activation(ksq[:], k_sb[:], AF.Square)
        nc.vector.reduce_sum(k2[:], ksq[:], axis=mybir.AxisListType.X)
        knorm = singles.tile([B, 1], f32)
        nc.scalar.activation(knorm[:], k2[:], AF.Sqrt)
        nc.vector.tensor_scalar_max(knorm[:], knorm[:], EPS_COS)

        # mem norms, den, cos
        norm_sb = singles.tile([B, N], f32)
        nc.scalar.activation(norm_sb[:], norms2_sb[:], AF.Sqrt)
        nc.vector.tensor_scalar_max(norm_sb[:], norm_sb[:], EPS_COS)
        den_sb = singles.tile([B, N], f32)
        nc.vector.tensor_scalar(den_sb[:], norm_sb[:], knorm[:, 0:1], None,
                                op0=AL.mult)
        rden_sb = singles.tile([B, N], f32)
        nc.vector.reciprocal(rden_sb[:], den_sb[:])
        cos_sb = singles.tile([B, N], f32)
        nc.vector.tensor_tensor(cos_sb[:], num_sb[:], rden_sb[:], AL.mult)

        # softmax(beta * cos): logits in (-1,1), no max-shift needed
        wc_sb = singles.tile([B, N], f32)
        sume = singles.tile([B, 1], f32)
        nc.scalar.activation(wc_sb[:], cos_sb[:], AF.Exp,
                             scale=beta_sb[:, 0:1])
        nc.vector.reduce_sum(sume[:], wc_sb[:], axis=mybir.AxisListType.X)
        rsume = singles.tile([B, 1], f32)
        nc.vector.reciprocal(rsume[:], sume[:])
        nc.vector.tensor_scalar(wc_sb[:], wc_sb[:], rsume[:, 0:1], None,
                                op0=AL.mult)

        # gate: w_g = g*w_c + (1-g)*w_prev
        omg = singles.tile([B, 1], f32)
        nc.vector.tensor_scalar(omg[:], g_sb[:], -1.0, 1.0,
                                op0=AL.mult, op1=AL.add)
        wg_sb = singles.tile([B, N], f32)
        nc.vector.tensor_scalar(wg_sb[:], wc_sb[:], g_sb[:, 0:1], None,
                                op0=AL.mult)
        nc.vector.scalar_tensor_tensor(
            out=wg_sb[:], in0=wprev_sb[:], scalar=omg[:, 0:1], in1=wg_sb[:],
            op0=AL.mult, op1=AL.add)

        # circular shift, kernel 3:
        # wt[i] = s0*wg[(i-1)%N] + s1*wg[i] + s2*wg[(i+1)%N]
        wt_sb = singles.tile([B, N], f32)
        s0, s1, s2 = s_sb[:, 0:1], s_sb[:, 1:2], s_sb[:, 2:3]
        nc.vector.tensor_scalar(wt_sb[:], wg_sb[:], s1, None, op0=AL.mult)
        nc.vector.scalar_tensor_tensor(
            out=wt_sb[:, 1:N], in0=wg_sb[:, 0:N - 1], scalar=s0,
            in1=wt_sb[:, 1:N], op0=AL.mult, op1=AL.add)
        nc.vector.scalar_tensor_tensor(
            out=wt_sb[:, 0:1], in0=wg_sb[:, N - 1:N], scalar=s0,
            in1=wt_sb[:, 0:1], op0=AL.mult, op1=AL.add)
        nc.vector.scalar_tensor_tensor(
            out=wt_sb[:, 0:N - 1], in0=wg_sb[:, 1:N], scalar=s2,
            in1=wt_sb[:, 0:N - 1], op0=AL.mult, op1=AL.add)
        nc.vector.scalar_tensor_tensor(
            out=wt_sb[:, N - 1:N], in0=wg_sb[:, 0:1], scalar=s2,
            in1=wt_sb[:, N - 1:N], op0=AL.mult, op1=AL.add)

        # sharpen: w = wt^gamma / (sum + eps);  wt^gamma = exp(gamma*ln(wt))
        ln_sb = singles.tile([B, N], f32)
        nc.scalar.activation(ln_sb[:], wt_sb[:], AF.Ln)
        nc.vector.tensor_scalar(ln_sb[:], ln_sb[:], gamma_sb[:, 0:1], None,
                                op0=AL.mult)
        wp_sb = singles.tile([B, N], f32)
        psum = singles.tile([B, 1], f32)
        nc.scalar.activation(wp_sb[:], ln_sb[:], AF.Exp)
        nc.vector.reduce_sum(psum[:], wp_sb[:], axis=mybir.AxisListType.X)
        nc.vector.tensor_scalar(psum[:], psum[:], EPS_ADD, None, op0=AL.add)
        rps = singles.tile([B, 1], f32)
        nc.vector.reciprocal(rps[:], psum[:])
        w_sb = singles.tile([B, N], f32)
        nc.vector.tensor_scalar(w_sb[:], wp_sb[:], rps[:, 0:1], None,
                                op0=AL.mult)
        negw_sb = singles.tile([B, N], f32)
        nc.vector.tensor_scalar(negw_sb[:], w_sb[:], -1.0, None, op0=AL.mult)

        # --- phase 2: read + write-back ------------------------------------
        r_sb = singles.tile([B, M], f32)
        nc.vector.memset(r_sb[:], 0.0)

        out3 = out_d[:, N + M:].rearrange("b (n m) -> b n m", m=M)
        for j in range(NSLABS):
            ms = mems.tile([B, SLAB, M], f32, tag="mem")
            nc.sync.dma_start(ms[:], mem_d[:, j * SLAB:(j + 1) * SLAB, :])
            fs = fpool.tile([B, SLAB, M], f32, tag="F")
            for t in range(SLAB):
                n = j * SLAB + t
                # F_n = 1 - w_n * e   (tensor_scalar, 2x fp32)
                nc.vector.tensor_scalar(
                    fs[:, t, :], e_sb[:], negw_sb[:, n:n + 1], 1.0,
                    op0=AL.mult, op1=AL.add)
            # v = mem * F  (slab-wide, in place into fs)
            nc.vector.tensor_tensor(fs[:], ms[:], fs[:], AL.mult)
            for t in range(SLAB):
                n = j * SLAB + t
                # r += w_n * mem_n
                nc.vector.scalar_tensor_tensor(
                    out=r_sb[:], in0=ms[:, t, :], scalar=w_sb[:, n:n + 1],
                    in1=r_sb[:], op0=AL.mult, op1=AL.add)
                # out_n = w_n * a + v_n   (in place into fs)
                nc.vector.scalar_tensor_tensor(
                    out=fs[:, t, :], in0=a_sb[:], scalar=w_sb[:, n:n + 1],
                    in1=fs[:, t, :], op0=AL.mult, op1=AL.add)
            nc.sync.dma_start(out3[:, j * SLAB:(j + 1) * SLAB, :], fs[:])

        nc.sync.dma_start(out_d[:, 0:N], w_sb[:])
        nc.sync.dma_start(out_d[:, N:N + M], r_sb[:])

    nc.compile()
    return nc


def kernel(**inputs) -> np.ndarray:
    global LAST_RESULTS
    from concourse.bass_utils import run_bass_kernel_spmd

    names = ["memory", "key", "beta", "g", "s", "gamma", "w_prev", "e", "a"]
    full = {k: np.ascontiguousarray(np.asarray(inputs[k], dtype=np.float32))
            for k in names}
    assert full["memory"].shape == (B * NCORES, N, M)

    in_maps = []
    for c in range(NCORES):
        sl = slice(c * B, (c + 1) * B)
        in_maps.append({k: np.ascontiguousarray(v[sl]) for k, v in full.items()})

    nc = _build()
    res = run_bass_kernel_spmd(nc, in_maps, core_ids=list(range(NCORES)))
    LAST_RESULTS = res
    return np.concatenate([r["out"] for r in res.results], axis=0)
